# revision 8
# baseline (speedup 1.0000x reference)
"""Expert-parallel MoE kernel for Trainium2 (8 NeuronCores).

Sharding: core e owns expert e. The host computes the top-2 routing and
the top-2 softmax gate weights (in float64; the gate is ~0.001% of the
FLOPs and is needed host-side anyway to build the shards). Each core runs
the heavy expert MLP relu(x@W1+b1)@W2+b2 on device in fp16 (fp32 PSUM
accumulation, rel err ~5e-4 vs the 2e-2 gate), scaled per token by the
host-supplied gate weight.

Device schedule: this expert's W1/W2 in fp16 total 128KB/partition, so
both live SBUF-resident for the whole launch (loaded once, outside the
timing-loop body). Tokens stream in blocks of TB; layer 2 runs
"transposed" (stationary = W2 chunk, moving = hT) so the output lands as
out^T [do, tok], and the gate weight (pre-broadcast across partitions on
host) is fused into the single DVE epilogue op per output tile. fp16
matmuls get hardware fast-weight-load; back-to-back N=TB fills hide the
weight loads entirely. The host transposes out^T back and scatter-adds
per-expert rows into the full [T, D_OUT] output.
"""

import math
import os
import sys

import numpy as np

sys.path.insert(0, "/opt/trn_rl_repo")

P = 128
E = 8
DIN = 1024
DH = 4096
DO = 1024
KC = DIN // P   # 8  k-chunks of x / W1 contraction
HC = DH // P    # 32 h-chunks of W2 contraction
DOC = DO // P   # 8  output chunks
NCORES = 8

_compiled = {}
LAST_DISPATCH_S = None


def _tb():
    return int(os.environ.get("MOE_TB", "512"))


def _build(blocks, reps):
    import concourse.mybir as mybir
    import concourse.tile as tile
    from concourse import bacc

    F32 = mybir.dt.float32
    MMDT = {"f32r": mybir.dt.float32r, "f16": mybir.dt.float16,
            "bf16": mybir.dt.bfloat16, "f32": mybir.dt.float32}[
        os.environ.get("MOE_DTYPE", "f16")]
    TB = _tb()

    cap = sum(blocks)

    nc = bacc.Bacc("TRN2", target_bir_lowering=False, debug=False,
                   num_devices=NCORES)

    xT = nc.dram_tensor("xT", [P, KC, cap], MMDT, kind="ExternalInput").ap()
    W1m = nc.dram_tensor("W1m", [P, KC, DH], MMDT, kind="ExternalInput").ap()
    W2m = nc.dram_tensor("W2m", [P, HC, DO], MMDT, kind="ExternalInput").ap()
    b1c = nc.dram_tensor("b1c", [P, HC], F32, kind="ExternalInput").ap()
    b2t = nc.dram_tensor("b2t", [P, DOC], F32, kind="ExternalInput").ap()
    wvm = nc.dram_tensor("wvm", [P, cap], F32, kind="ExternalInput").ap()
    # transposed output: out^T[doc, p, t] = out[t, doc*128+p]
    outT = nc.dram_tensor("outT", [DOC, P, cap], F32, kind="ExternalOutput").ap()

    with tile.TileContext(nc) as tc:
        with tc.tile_pool(name="const", bufs=1) as cpool, \
             tc.tile_pool(name="xtp", bufs=2) as xtp, \
             tc.tile_pool(name="htp", bufs=1) as htp, \
             tc.tile_pool(name="obp", bufs=4) as obp, \
             tc.tile_pool(name="ps", bufs=4, space="PSUM") as ps:

            b1_sb = cpool.tile([P, HC], F32)
            nc.sync.dma_start(b1_sb[:], b1c[:])
            b2_sb = cpool.tile([P, DOC], F32)
            nc.sync.dma_start(b2_sb[:], b2t[:])
            wv_sb = cpool.tile([P, cap], F32)
            nc.sync.dma_start(wv_sb[:], wvm[:])
            # this expert's weights, SBUF-resident for the whole launch
            w1_sb = cpool.tile([P, KC, DH], MMDT)
            nc.sync.dma_start(w1_sb[:], W1m[:])
            w2_sb = cpool.tile([P, HC, DO], MMDT)
            nc.sync.dma_start(w2_sb[:], W2m[:])

            def body(_iv=None):
                t0 = 0
                for tb in blocks:
                    xt = xtp.tile([P, KC, TB], MMDT, tag="xt",
                                  name="xt")[:, :, :tb]
                    nc.sync.dma_start(xt[:], xT[:, :, t0:t0 + tb])

                    # layer 1: hT[hc] = relu(W1[:, :, hc].T @ x + b1[hc])
                    hT = htp.tile([P, HC, TB], MMDT, tag="hT",
                                  name="hT")[:, :, :tb]
                    for hc in range(HC):
                        ps1 = ps.tile([P, TB], F32, tag="mm",
                                      name="mm")[:, :tb]
                        for kc in range(KC):
                            nc.tensor.matmul(
                                ps1[:], w1_sb[:, kc, hc * P:(hc + 1) * P],
                                xt[:, kc, :],
                                start=(kc == 0), stop=(kc == KC - 1))
                        nc.scalar.activation(
                            hT[:, hc, :], ps1[:],
                            mybir.ActivationFunctionType.Relu,
                            bias=b1_sb[:, hc:hc + 1], scale=1.0)

                    # layer 2 (transposed): outT[doc] = W2[:, doc].T @ hT
                    for doc in range(DOC):
                        ps2 = ps.tile([P, TB], F32, tag="mm",
                                      name="mm")[:, :tb]
                        for hc in range(HC):
                            nc.tensor.matmul(
                                ps2[:], w2_sb[:, hc, doc * P:(doc + 1) * P],
                                hT[:, hc, :],
                                start=(hc == 0), stop=(hc == HC - 1))
                        ob = obp.tile([P, TB], F32, tag="ob",
                                      name="ob")[:, :tb]
                        nc.vector.scalar_tensor_tensor(
                            ob[:], ps2[:], b2_sb[:, doc:doc + 1],
                            wv_sb[:, t0:t0 + tb],
                            mybir.AluOpType.add, mybir.AluOpType.mult)
                        nc.sync.dma_start(outT[doc, :, t0:t0 + tb], ob[:])
                    t0 += tb

            if reps > 1:
                with tc.For_i(0, reps, 1) as _i:
                    body(_i)
            else:
                body()

    nc.compile()
    return nc


def _get_compiled(blocks, reps):
    key = (tuple(blocks), reps, os.environ.get("MOE_DTYPE", "f16"), _tb())
    if key not in _compiled:
        _compiled[key] = _build(blocks, reps)
    return _compiled[key]


def kernel(x, Wg, bg, W1, b1, W2, b2):
    import time as _time

    from concourse.bass_utils import run_bass_kernel_spmd

    x = np.ascontiguousarray(np.asarray(x, dtype=np.float32))
    Wg = np.ascontiguousarray(np.asarray(Wg, dtype=np.float32))
    bg = np.ascontiguousarray(np.asarray(bg, dtype=np.float32))
    W1 = np.ascontiguousarray(np.asarray(W1, dtype=np.float32))
    b1 = np.ascontiguousarray(np.asarray(b1, dtype=np.float32))
    W2 = np.ascontiguousarray(np.asarray(W2, dtype=np.float32))
    b2 = np.ascontiguousarray(np.asarray(b2, dtype=np.float32))

    T = x.shape[0]
    TB = _tb()

    # Host-side routing + top-2 softmax gate weights (float64).
    logits = x.astype(np.float64) @ Wg.astype(np.float64) + bg.astype(np.float64)
    top2 = np.argpartition(logits, -2, axis=1)[:, -2:]
    l2 = np.take_along_axis(logits, top2, axis=1)          # [T, 2]
    m = l2.max(axis=1, keepdims=True)
    e2 = np.exp(l2 - m)
    sm = e2 / e2.sum(axis=1, keepdims=True)                # [T, 2]
    gatew = np.zeros((T, E), dtype=np.float64)
    gatew[np.arange(T)[:, None], top2] = sm                # dense [T, E]
    sel_mask = gatew > 0.0

    idx_e = [np.nonzero(sel_mask[:, e])[0] for e in range(E)]
    counts = [len(i) for i in idx_e]
    cap = max(P, int(math.ceil(max(counts) / P)) * P)
    nfull, rem = divmod(cap, TB)
    blocks = [TB] * nfull + ([rem] if rem else [])

    reps = int(os.environ.get("MOE_REPS", "1"))
    nc = _get_compiled(blocks, reps)

    import ml_dtypes
    npdt = {"f32r": np.float32, "f32": np.float32,
            "f16": np.float16, "bf16": ml_dtypes.bfloat16}[
        os.environ.get("MOE_DTYPE", "f16")]

    in_maps = []
    for e in range(E):
        n = counts[e]
        xe = np.zeros((cap, DIN), dtype=np.float32)
        xe[:n] = x[idx_e[e]]
        wv = np.zeros(cap, dtype=np.float32)
        wv[:n] = gatew[idx_e[e], e].astype(np.float32)
        in_maps.append({
            "xT": np.ascontiguousarray(
                xe.T.reshape(KC, P, cap).transpose(1, 0, 2).astype(npdt)),
            "W1m": np.ascontiguousarray(
                W1[e].reshape(KC, P, DH).transpose(1, 0, 2).astype(npdt)),
            "W2m": np.ascontiguousarray(
                W2[e].reshape(HC, P, DO).transpose(1, 0, 2).astype(npdt)),
            "b1c": np.ascontiguousarray(b1[e].reshape(HC, P).T),
            "b2t": np.ascontiguousarray(b2[e].reshape(DOC, P).T),
            "wvm": np.ascontiguousarray(np.tile(wv, (P, 1))),
        })

    _t0 = _time.time()
    res = run_bass_kernel_spmd(nc, in_maps, list(range(NCORES)))
    global LAST_DISPATCH_S
    LAST_DISPATCH_S = _time.time() - _t0

    outf = np.zeros((T, DO), dtype=np.float32)
    for e in range(E):
        oT = res.results[e]["outT"]                  # [DOC, P, cap]
        oe = oT.transpose(2, 0, 1).reshape(cap, DO)  # [cap, DO]
        outf[idx_e[e]] += oe[:counts[e]]
    return outf


# revision 12
# speedup vs baseline: 1.0608x; 1.0608x over previous
"""Expert-parallel MoE kernel for Trainium2 (8 NeuronCores).

Sharding: core e owns expert e. The host computes the top-2 routing and
the top-2 softmax gate weights (in float64; the gate is ~0.001% of the
FLOPs and is needed host-side anyway to build the shards). Each core runs
the heavy expert MLP relu(x@W1+b1)@W2+b2 on device in fp16 (fp32 PSUM
accumulation, rel err ~5e-4 vs the 2e-2 gate), scaled per token by the
host-supplied gate weight.

Device schedule: this expert's W1/W2 in fp16 total 128KB/partition, so
both live SBUF-resident for the whole launch (loaded once, outside the
timing-loop body). Tokens stream in blocks of TB; layer 2 runs
"transposed" (stationary = W2 chunk, moving = hT) so the output lands as
out^T [do, tok], and the gate weight (pre-broadcast across partitions on
host) is fused into the single DVE epilogue op per output tile. fp16
matmuls get hardware fast-weight-load; back-to-back N=TB fills hide the
weight loads entirely. The host transposes out^T back and scatter-adds
per-expert rows into the full [T, D_OUT] output.
"""

import math
import os
import sys

import numpy as np

sys.path.insert(0, "/opt/trn_rl_repo")

P = 128
E = 8
DIN = 1024
DH = 4096
DO = 1024
KC = DIN // P   # 8  k-chunks of x / W1 contraction
HC = DH // P    # 32 h-chunks of W2 contraction
DOC = DO // P   # 8  output chunks
NCORES = 8

_compiled = {}
LAST_DISPATCH_S = None


def _tb():
    return int(os.environ.get("MOE_TB", "512"))


def _build(blocks, reps):
    import concourse.mybir as mybir
    import concourse.tile as tile
    from concourse import bacc

    F32 = mybir.dt.float32
    MMDT = {"f32r": mybir.dt.float32r, "f16": mybir.dt.float16,
            "bf16": mybir.dt.bfloat16, "f32": mybir.dt.float32}[
        os.environ.get("MOE_DTYPE", "f16")]
    TB = _tb()

    cap = sum(blocks)

    nc = bacc.Bacc("TRN2", target_bir_lowering=False, debug=False,
                   num_devices=NCORES)

    xT = nc.dram_tensor("xT", [P, KC, cap], MMDT, kind="ExternalInput").ap()
    W1m = nc.dram_tensor("W1m", [P, KC, DH], MMDT, kind="ExternalInput").ap()
    W2m = nc.dram_tensor("W2m", [P, HC, DO], MMDT, kind="ExternalInput").ap()
    b1c = nc.dram_tensor("b1c", [P, HC], F32, kind="ExternalInput").ap()
    b2t = nc.dram_tensor("b2t", [P, DOC], F32, kind="ExternalInput").ap()
    wvm = nc.dram_tensor("wvm", [P, cap], F32, kind="ExternalInput").ap()
    # transposed output: out^T[doc, p, t] = out[t, doc*128+p]
    outT = nc.dram_tensor("outT", [DOC, P, cap], F32, kind="ExternalOutput").ap()

    PSB = int(os.environ.get("MOE_PSB", "8"))
    BIAS = os.environ.get("MOE_BIAS", "1") == "1"

    with tile.TileContext(nc) as tc:
        with tc.tile_pool(name="const", bufs=1) as cpool, \
             tc.tile_pool(name="xtp", bufs=2) as xtp, \
             tc.tile_pool(name="htp", bufs=1) as htp, \
             tc.tile_pool(name="obp", bufs=4) as obp, \
             tc.tile_pool(name="ps", bufs=PSB, space="PSUM") as ps:

            b1_sb = cpool.tile([P, HC], F32)
            nc.sync.dma_start(b1_sb[:], b1c[:])
            b2_sb = cpool.tile([P, DOC], F32)
            nc.sync.dma_start(b2_sb[:], b2t[:])
            wv_sb = cpool.tile([P, cap], F32)
            nc.sync.dma_start(wv_sb[:], wvm[:])
            # this expert's weights, SBUF-resident for the whole launch
            w1_sb = cpool.tile([P, KC, DH], MMDT)
            nc.sync.dma_start(w1_sb[:], W1m[:])
            w2_sb = cpool.tile([P, HC, DO], MMDT)
            nc.sync.dma_start(w2_sb[:], W2m[:])

            def body(_iv=None):
                t0 = 0
                for tb in blocks:
                    xt = xtp.tile([P, KC, TB], MMDT, tag="xt",
                                  name="xt")[:, :, :tb]
                    nc.sync.dma_start(xt[:], xT[:, :, t0:t0 + tb])

                    # layer 1: hT[hc] = relu(W1[:, :, hc].T @ x + b1[hc])
                    hT = htp.tile([P, HC, TB], MMDT, tag="hT",
                                  name="hT")[:, :, :tb]
                    for hc in range(HC):
                        ps1 = ps.tile([P, TB], F32, tag="mm",
                                      name="mm")[:, :tb]
                        for kc in range(KC):
                            nc.tensor.matmul(
                                ps1[:], w1_sb[:, kc, hc * P:(hc + 1) * P],
                                xt[:, kc, :],
                                start=(kc == 0), stop=(kc == KC - 1))
                        if BIAS:
                            nc.scalar.activation(
                                hT[:, hc, :], ps1[:],
                                mybir.ActivationFunctionType.Relu,
                                bias=b1_sb[:, hc:hc + 1], scale=1.0)
                        else:
                            nc.scalar.activation(
                                hT[:, hc, :], ps1[:],
                                mybir.ActivationFunctionType.Relu)

                    # layer 2 (transposed): outT[doc] = W2[:, doc].T @ hT
                    for doc in range(DOC):
                        ps2 = ps.tile([P, TB], F32, tag="mm",
                                      name="mm")[:, :tb]
                        for hc in range(HC):
                            nc.tensor.matmul(
                                ps2[:], w2_sb[:, hc, doc * P:(doc + 1) * P],
                                hT[:, hc, :],
                                start=(hc == 0), stop=(hc == HC - 1))
                        ob = obp.tile([P, TB], F32, tag="ob",
                                      name="ob")[:, :tb]
                        if BIAS:
                            nc.vector.scalar_tensor_tensor(
                                ob[:], ps2[:], b2_sb[:, doc:doc + 1],
                                wv_sb[:, t0:t0 + tb],
                                mybir.AluOpType.add, mybir.AluOpType.mult)
                        else:
                            nc.vector.tensor_tensor(
                                ob[:], ps2[:], wv_sb[:, t0:t0 + tb],
                                mybir.AluOpType.mult)
                        nc.sync.dma_start(outT[doc, :, t0:t0 + tb], ob[:])
                    t0 += tb

            if reps > 1:
                with tc.For_i(0, reps, 1) as _i:
                    body(_i)
            else:
                body()

    nc.compile()
    return nc


def _get_compiled(blocks, reps):
    key = (tuple(blocks), reps, os.environ.get("MOE_DTYPE", "f16"), _tb(),
           os.environ.get("MOE_PSB", "8"), os.environ.get("MOE_BIAS", "1"))
    if key not in _compiled:
        _compiled[key] = _build(blocks, reps)
    return _compiled[key]


def kernel(x, Wg, bg, W1, b1, W2, b2):
    import time as _time

    from concourse.bass_utils import run_bass_kernel_spmd

    x = np.ascontiguousarray(np.asarray(x, dtype=np.float32))
    Wg = np.ascontiguousarray(np.asarray(Wg, dtype=np.float32))
    bg = np.ascontiguousarray(np.asarray(bg, dtype=np.float32))
    W1 = np.ascontiguousarray(np.asarray(W1, dtype=np.float32))
    b1 = np.ascontiguousarray(np.asarray(b1, dtype=np.float32))
    W2 = np.ascontiguousarray(np.asarray(W2, dtype=np.float32))
    b2 = np.ascontiguousarray(np.asarray(b2, dtype=np.float32))

    T = x.shape[0]
    TB = _tb()

    # Host-side routing + top-2 softmax gate weights (float64).
    logits = x.astype(np.float64) @ Wg.astype(np.float64) + bg.astype(np.float64)
    top2 = np.argpartition(logits, -2, axis=1)[:, -2:]
    l2 = np.take_along_axis(logits, top2, axis=1)          # [T, 2]
    m = l2.max(axis=1, keepdims=True)
    e2 = np.exp(l2 - m)
    sm = e2 / e2.sum(axis=1, keepdims=True)                # [T, 2]
    gatew = np.zeros((T, E), dtype=np.float64)
    gatew[np.arange(T)[:, None], top2] = sm                # dense [T, E]
    sel_mask = gatew > 0.0

    idx_e = [np.nonzero(sel_mask[:, e])[0] for e in range(E)]
    counts = [len(i) for i in idx_e]
    cap = max(P, int(math.ceil(max(counts) / P)) * P)
    nfull, rem = divmod(cap, TB)
    blocks = [TB] * nfull + ([rem] if rem else [])

    reps = int(os.environ.get("MOE_REPS", "1"))
    nc = _get_compiled(blocks, reps)

    import ml_dtypes
    npdt = {"f32r": np.float32, "f32": np.float32,
            "f16": np.float16, "bf16": ml_dtypes.bfloat16}[
        os.environ.get("MOE_DTYPE", "f16")]

    in_maps = []
    for e in range(E):
        n = counts[e]
        xe = np.zeros((cap, DIN), dtype=np.float32)
        xe[:n] = x[idx_e[e]]
        wv = np.zeros(cap, dtype=np.float32)
        wv[:n] = gatew[idx_e[e], e].astype(np.float32)
        in_maps.append({
            "xT": np.ascontiguousarray(
                xe.T.reshape(KC, P, cap).transpose(1, 0, 2).astype(npdt)),
            "W1m": np.ascontiguousarray(
                W1[e].reshape(KC, P, DH).transpose(1, 0, 2).astype(npdt)),
            "W2m": np.ascontiguousarray(
                W2[e].reshape(HC, P, DO).transpose(1, 0, 2).astype(npdt)),
            "b1c": np.ascontiguousarray(b1[e].reshape(HC, P).T),
            "b2t": np.ascontiguousarray(b2[e].reshape(DOC, P).T),
            "wvm": np.ascontiguousarray(np.tile(wv, (P, 1))),
        })

    _t0 = _time.time()
    res = run_bass_kernel_spmd(nc, in_maps, list(range(NCORES)))
    global LAST_DISPATCH_S
    LAST_DISPATCH_S = _time.time() - _t0

    outf = np.zeros((T, DO), dtype=np.float32)
    for e in range(E):
        oT = res.results[e]["outT"]                  # [DOC, P, cap]
        oe = oT.transpose(2, 0, 1).reshape(cap, DO)  # [cap, DO]
        outf[idx_e[e]] += oe[:counts[e]]
    return outf


# revision 16
# speedup vs baseline: 1.1721x; 1.1049x over previous
"""Expert-parallel MoE kernel for Trainium2 (8 NeuronCores).

Sharding: core e owns expert e. The host computes the top-2 routing and
the top-2 softmax gate weights (in float64; the gate is ~0.001% of the
FLOPs and is needed host-side anyway to build the shards). Each core runs
the heavy expert MLP relu(x@W1+b1)@W2+b2 on device in fp16 (fp32 PSUM
accumulation, rel err ~5e-4 vs the 2e-2 gate), scaled per token by the
host-supplied gate weight.

Device schedule: this expert's W1/W2 in fp16 total 128KB/partition, so
both live SBUF-resident for the whole launch (loaded once, outside the
timing-loop body). Tokens stream in blocks of TB; layer 2 runs
"transposed" (stationary = W2 chunk, moving = hT) so the output lands as
out^T [do, tok], and the gate weight (pre-broadcast across partitions on
host) is fused into the single DVE epilogue op per output tile. fp16
matmuls get hardware fast-weight-load; back-to-back N=TB fills hide the
weight loads entirely. The host transposes out^T back and scatter-adds
per-expert rows into the full [T, D_OUT] output.
"""

import math
import os
import sys

import numpy as np

sys.path.insert(0, "/opt/trn_rl_repo")

P = 128
E = 8
DIN = 1024
DH = 4096
DO = 1024
KC = DIN // P   # 8  k-chunks of x / W1 contraction
HC = DH // P    # 32 h-chunks of W2 contraction
DOC = DO // P   # 8  output chunks
NCORES = 8

_compiled = {}
LAST_DISPATCH_S = None


def _tb():
    return int(os.environ.get("MOE_TB", "512"))


def _build(blocks, reps):
    import concourse.mybir as mybir
    import concourse.tile as tile
    from concourse import bacc

    F32 = mybir.dt.float32
    MMDT = {"f32r": mybir.dt.float32r, "f16": mybir.dt.float16,
            "bf16": mybir.dt.bfloat16, "f32": mybir.dt.float32}[
        os.environ.get("MOE_DTYPE", "f16")]
    TB = _tb()

    cap = sum(blocks)

    nc = bacc.Bacc("TRN2", target_bir_lowering=False, debug=False,
                   num_devices=NCORES)

    xT = nc.dram_tensor("xT", [P, KC, cap], MMDT, kind="ExternalInput").ap()
    W1m = nc.dram_tensor("W1m", [P, KC, DH], MMDT, kind="ExternalInput").ap()
    W2m = nc.dram_tensor("W2m", [P, HC, DO], MMDT, kind="ExternalInput").ap()
    b1c = nc.dram_tensor("b1c", [P, HC], F32, kind="ExternalInput").ap()
    b2t = nc.dram_tensor("b2t", [P, DOC], F32, kind="ExternalInput").ap()
    wvm = nc.dram_tensor("wvm", [P, cap], F32, kind="ExternalInput").ap()
    ODT = F32 if os.environ.get("MOE_OUTF32") else mybir.dt.float16
    # transposed output: out2[p, doc, t] = out[t, doc*128+p]
    out2 = nc.dram_tensor("out2", [P, DOC, cap], ODT, kind="ExternalOutput").ap()

    PSB = int(os.environ.get("MOE_PSB", "8"))
    BIAS = os.environ.get("MOE_BIAS", "1") == "1"

    with tile.TileContext(nc) as tc:
        with tc.tile_pool(name="const", bufs=1) as cpool, \
             tc.tile_pool(name="xtp", bufs=2) as xtp, \
             tc.tile_pool(name="htp", bufs=1) as htp, \
             tc.tile_pool(name="obp", bufs=2) as obp, \
             tc.tile_pool(name="ps", bufs=PSB, space="PSUM") as ps:

            b1_sb = cpool.tile([P, HC], F32)
            nc.sync.dma_start(b1_sb[:], b1c[:])
            b2_sb = cpool.tile([P, DOC], F32)
            nc.sync.dma_start(b2_sb[:], b2t[:])
            wv_sb = cpool.tile([P, cap], F32)
            nc.sync.dma_start(wv_sb[:], wvm[:])
            # this expert's weights, SBUF-resident for the whole launch
            w1_sb = cpool.tile([P, KC, DH], MMDT)
            nc.sync.dma_start(w1_sb[:], W1m[:])
            w2_sb = cpool.tile([P, HC, DO], MMDT)
            nc.sync.dma_start(w2_sb[:], W2m[:])

            def body(_iv=None):
                t0 = 0
                for tb in blocks:
                    xt = xtp.tile([P, KC, TB], MMDT, tag="xt",
                                  name="xt")[:, :, :tb]
                    nc.sync.dma_start(xt[:], xT[:, :, t0:t0 + tb])

                    # layer 1: hT[hc] = relu(W1[:, :, hc].T @ x + b1[hc])
                    hT = htp.tile([P, HC, TB], MMDT, tag="hT",
                                  name="hT")[:, :, :tb]
                    for hc in range(HC):
                        ps1 = ps.tile([P, TB], F32, tag="mm",
                                      name="mm")[:, :tb]
                        for kc in range(KC):
                            nc.tensor.matmul(
                                ps1[:], w1_sb[:, kc, hc * P:(hc + 1) * P],
                                xt[:, kc, :],
                                start=(kc == 0), stop=(kc == KC - 1))
                        if BIAS:
                            nc.scalar.activation(
                                hT[:, hc, :], ps1[:],
                                mybir.ActivationFunctionType.Relu,
                                bias=b1_sb[:, hc:hc + 1], scale=1.0)
                        else:
                            nc.scalar.activation(
                                hT[:, hc, :], ps1[:],
                                mybir.ActivationFunctionType.Relu)

                    # layer 2 (transposed): out2[:, doc] = W2[:, doc].T @ hT,
                    # staged per block, one batched store
                    ob = obp.tile([P, DOC, TB], ODT, tag="ob",
                                  name="ob")[:, :, :tb]
                    for doc in range(DOC):
                        ps2 = ps.tile([P, TB], F32, tag="mm",
                                      name="mm")[:, :tb]
                        for hc in range(HC):
                            nc.tensor.matmul(
                                ps2[:], w2_sb[:, hc, doc * P:(doc + 1) * P],
                                hT[:, hc, :],
                                start=(hc == 0), stop=(hc == HC - 1))
                        if BIAS:
                            nc.vector.scalar_tensor_tensor(
                                ob[:, doc, :], ps2[:], b2_sb[:, doc:doc + 1],
                                wv_sb[:, t0:t0 + tb],
                                mybir.AluOpType.add, mybir.AluOpType.mult)
                        else:
                            nc.vector.tensor_tensor(
                                ob[:, doc, :], ps2[:], wv_sb[:, t0:t0 + tb],
                                mybir.AluOpType.mult)
                    nc.sync.dma_start(out2[:, :, t0:t0 + tb], ob[:])
                    t0 += tb

            if reps > 1:
                with tc.For_i(0, reps, 1) as _i:
                    body(_i)
            else:
                body()

    nc.compile()
    return nc


def _get_compiled(blocks, reps):
    key = (tuple(blocks), reps, os.environ.get("MOE_DTYPE", "f16"), _tb(),
           os.environ.get("MOE_PSB", "8"), os.environ.get("MOE_BIAS", "1"))
    if key not in _compiled:
        _compiled[key] = _build(blocks, reps)
    return _compiled[key]


def kernel(x, Wg, bg, W1, b1, W2, b2):
    import time as _time

    from concourse.bass_utils import run_bass_kernel_spmd

    x = np.ascontiguousarray(np.asarray(x, dtype=np.float32))
    Wg = np.ascontiguousarray(np.asarray(Wg, dtype=np.float32))
    bg = np.ascontiguousarray(np.asarray(bg, dtype=np.float32))
    W1 = np.ascontiguousarray(np.asarray(W1, dtype=np.float32))
    b1 = np.ascontiguousarray(np.asarray(b1, dtype=np.float32))
    W2 = np.ascontiguousarray(np.asarray(W2, dtype=np.float32))
    b2 = np.ascontiguousarray(np.asarray(b2, dtype=np.float32))

    T = x.shape[0]
    TB = _tb()

    # Host-side routing + top-2 softmax gate weights (float64).
    logits = x.astype(np.float64) @ Wg.astype(np.float64) + bg.astype(np.float64)
    top2 = np.argpartition(logits, -2, axis=1)[:, -2:]
    l2 = np.take_along_axis(logits, top2, axis=1)          # [T, 2]
    m = l2.max(axis=1, keepdims=True)
    e2 = np.exp(l2 - m)
    sm = e2 / e2.sum(axis=1, keepdims=True)                # [T, 2]
    gatew = np.zeros((T, E), dtype=np.float64)
    gatew[np.arange(T)[:, None], top2] = sm                # dense [T, E]
    sel_mask = gatew > 0.0

    idx_e = [np.nonzero(sel_mask[:, e])[0] for e in range(E)]
    counts = [len(i) for i in idx_e]
    cap = max(P, int(math.ceil(max(counts) / P)) * P)
    nfull, rem = divmod(cap, TB)
    blocks = [TB] * nfull + ([rem] if rem else [])

    reps = int(os.environ.get("MOE_REPS", "1"))
    nc = _get_compiled(blocks, reps)

    import ml_dtypes
    npdt = {"f32r": np.float32, "f32": np.float32,
            "f16": np.float16, "bf16": ml_dtypes.bfloat16}[
        os.environ.get("MOE_DTYPE", "f16")]

    in_maps = []
    for e in range(E):
        n = counts[e]
        xe = np.zeros((cap, DIN), dtype=np.float32)
        xe[:n] = x[idx_e[e]]
        wv = np.zeros(cap, dtype=np.float32)
        wv[:n] = gatew[idx_e[e], e].astype(np.float32)
        in_maps.append({
            "xT": np.ascontiguousarray(
                xe.T.reshape(KC, P, cap).transpose(1, 0, 2).astype(npdt)),
            "W1m": np.ascontiguousarray(
                W1[e].reshape(KC, P, DH).transpose(1, 0, 2).astype(npdt)),
            "W2m": np.ascontiguousarray(
                W2[e].reshape(HC, P, DO).transpose(1, 0, 2).astype(npdt)),
            "b1c": np.ascontiguousarray(b1[e].reshape(HC, P).T),
            "b2t": np.ascontiguousarray(b2[e].reshape(DOC, P).T),
            "wvm": np.ascontiguousarray(np.tile(wv, (P, 1))),
        })

    _t0 = _time.time()
    res = run_bass_kernel_spmd(nc, in_maps, list(range(NCORES)))
    global LAST_DISPATCH_S
    LAST_DISPATCH_S = _time.time() - _t0

    outf = np.zeros((T, DO), dtype=np.float32)
    for e in range(E):
        o2 = res.results[e]["out2"]                  # [P, DOC, cap]
        oe = (o2.transpose(2, 1, 0).astype(np.float32)
              .reshape(cap, DO))                     # [cap, DO]
        outf[idx_e[e]] += oe[:counts[e]]
    return outf


# revision 17
# speedup vs baseline: 1.1907x; 1.0159x over previous
"""Expert-parallel MoE kernel for Trainium2 (8 NeuronCores).

Sharding: core e owns expert e. The host computes the top-2 routing and
the top-2 softmax gate weights (in float64; the gate is ~0.001% of the
FLOPs and is needed host-side anyway to build the shards). Each core runs
the heavy expert MLP relu(x@W1+b1)@W2+b2 on device in fp16 (fp32 PSUM
accumulation, rel err ~5e-4 vs the 2e-2 gate), scaled per token by the
host-supplied gate weight.

Device schedule: this expert's W1/W2 in fp16 total 128KB/partition, so
both live SBUF-resident for the whole launch (loaded once, outside the
timing-loop body). Tokens stream in blocks of TB=512; layer 2 runs
"transposed" (stationary = W2 chunk, moving = hT) so the output lands as
out2[p, doc, tok] = out[tok, doc*128+p], and the gate weight
(pre-broadcast across partitions on host) is fused into the single DVE
epilogue op per output tile, written in fp16 into a per-block staging
tile that is stored with ONE batched DMA per block (40 -> 5 stores per
pass; the batched fp16 store measured ~270us faster than per-tile fp32
stores). fp16 matmuls get hardware fast-weight-load; back-to-back N=512
fills hide the weight loads entirely (measured 224 ns/MM vs the 213 ns
fill floor). The host transposes out2 back and scatter-adds per-expert
rows into the full [T, D_OUT] output.
"""

import math
import os
import sys

import numpy as np

sys.path.insert(0, "/opt/trn_rl_repo")

P = 128
E = 8
DIN = 1024
DH = 4096
DO = 1024
KC = DIN // P   # 8  k-chunks of x / W1 contraction
HC = DH // P    # 32 h-chunks of W2 contraction
DOC = DO // P   # 8  output chunks
NCORES = 8

_compiled = {}
LAST_DISPATCH_S = None


def _tb():
    return int(os.environ.get("MOE_TB", "512"))


def _build(blocks, reps):
    import concourse.mybir as mybir
    import concourse.tile as tile
    from concourse import bacc

    F32 = mybir.dt.float32
    MMDT = {"f32r": mybir.dt.float32r, "f16": mybir.dt.float16,
            "bf16": mybir.dt.bfloat16, "f32": mybir.dt.float32}[
        os.environ.get("MOE_DTYPE", "f16")]
    TB = _tb()

    cap = sum(blocks)

    nc = bacc.Bacc("TRN2", target_bir_lowering=False, debug=False,
                   num_devices=NCORES)

    xT = nc.dram_tensor("xT", [P, KC, cap], MMDT, kind="ExternalInput").ap()
    W1m = nc.dram_tensor("W1m", [P, KC, DH], MMDT, kind="ExternalInput").ap()
    W2m = nc.dram_tensor("W2m", [P, HC, DO], MMDT, kind="ExternalInput").ap()
    b1c = nc.dram_tensor("b1c", [P, HC], F32, kind="ExternalInput").ap()
    b2t = nc.dram_tensor("b2t", [P, DOC], F32, kind="ExternalInput").ap()
    wvm = nc.dram_tensor("wvm", [P, cap], F32, kind="ExternalInput").ap()
    ODT = F32 if os.environ.get("MOE_OUTF32") else mybir.dt.float16
    # transposed output: out2[p, doc, t] = out[t, doc*128+p]
    out2 = nc.dram_tensor("out2", [P, DOC, cap], ODT, kind="ExternalOutput").ap()

    PSB = int(os.environ.get("MOE_PSB", "8"))
    BIAS = os.environ.get("MOE_BIAS", "1") == "1"

    with tile.TileContext(nc) as tc:
        with tc.tile_pool(name="const", bufs=1) as cpool, \
             tc.tile_pool(name="xtp", bufs=2) as xtp, \
             tc.tile_pool(name="htp", bufs=1) as htp, \
             tc.tile_pool(name="obp", bufs=2) as obp, \
             tc.tile_pool(name="ps", bufs=PSB, space="PSUM") as ps:

            b1_sb = cpool.tile([P, HC], F32)
            nc.sync.dma_start(b1_sb[:], b1c[:])
            b2_sb = cpool.tile([P, DOC], F32)
            nc.sync.dma_start(b2_sb[:], b2t[:])
            wv_sb = cpool.tile([P, cap], F32)
            nc.sync.dma_start(wv_sb[:], wvm[:])
            # this expert's weights, SBUF-resident for the whole launch
            w1_sb = cpool.tile([P, KC, DH], MMDT)
            nc.sync.dma_start(w1_sb[:], W1m[:])
            w2_sb = cpool.tile([P, HC, DO], MMDT)
            nc.sync.dma_start(w2_sb[:], W2m[:])

            def body(_iv=None):
                t0 = 0
                for tb in blocks:
                    xt = xtp.tile([P, KC, TB], MMDT, tag="xt",
                                  name="xt")[:, :, :tb]
                    nc.sync.dma_start(xt[:], xT[:, :, t0:t0 + tb])

                    # layer 1: hT[hc] = relu(W1[:, :, hc].T @ x + b1[hc])
                    hT = htp.tile([P, HC, TB], MMDT, tag="hT",
                                  name="hT")[:, :, :tb]
                    for hc in range(HC):
                        ps1 = ps.tile([P, TB], F32, tag="mm",
                                      name="mm")[:, :tb]
                        for kc in range(KC):
                            nc.tensor.matmul(
                                ps1[:], w1_sb[:, kc, hc * P:(hc + 1) * P],
                                xt[:, kc, :],
                                start=(kc == 0), stop=(kc == KC - 1))
                        if BIAS:
                            nc.scalar.activation(
                                hT[:, hc, :], ps1[:],
                                mybir.ActivationFunctionType.Relu,
                                bias=b1_sb[:, hc:hc + 1], scale=1.0)
                        else:
                            nc.scalar.activation(
                                hT[:, hc, :], ps1[:],
                                mybir.ActivationFunctionType.Relu)

                    # layer 2 (transposed): out2[:, doc] = W2[:, doc].T @ hT,
                    # staged per block, one batched store
                    ob = obp.tile([P, DOC, TB], ODT, tag="ob",
                                  name="ob")[:, :, :tb]
                    for doc in range(DOC):
                        ps2 = ps.tile([P, TB], F32, tag="mm",
                                      name="mm")[:, :tb]
                        for hc in range(HC):
                            nc.tensor.matmul(
                                ps2[:], w2_sb[:, hc, doc * P:(doc + 1) * P],
                                hT[:, hc, :],
                                start=(hc == 0), stop=(hc == HC - 1))
                        if BIAS:
                            nc.vector.scalar_tensor_tensor(
                                ob[:, doc, :], ps2[:], b2_sb[:, doc:doc + 1],
                                wv_sb[:, t0:t0 + tb],
                                mybir.AluOpType.add, mybir.AluOpType.mult)
                        else:
                            nc.vector.tensor_tensor(
                                ob[:, doc, :], ps2[:], wv_sb[:, t0:t0 + tb],
                                mybir.AluOpType.mult)
                    nc.sync.dma_start(out2[:, :, t0:t0 + tb], ob[:])
                    t0 += tb

            if reps > 1:
                with tc.For_i(0, reps, 1) as _i:
                    body(_i)
            else:
                body()

    nc.compile()
    return nc


def _get_compiled(blocks, reps):
    key = (tuple(blocks), reps, os.environ.get("MOE_DTYPE", "f16"), _tb(),
           os.environ.get("MOE_PSB", "8"), os.environ.get("MOE_BIAS", "1"))
    if key not in _compiled:
        _compiled[key] = _build(blocks, reps)
    return _compiled[key]


def kernel(x, Wg, bg, W1, b1, W2, b2):
    import time as _time

    from concourse.bass_utils import run_bass_kernel_spmd

    x = np.ascontiguousarray(np.asarray(x, dtype=np.float32))
    Wg = np.ascontiguousarray(np.asarray(Wg, dtype=np.float32))
    bg = np.ascontiguousarray(np.asarray(bg, dtype=np.float32))
    W1 = np.ascontiguousarray(np.asarray(W1, dtype=np.float32))
    b1 = np.ascontiguousarray(np.asarray(b1, dtype=np.float32))
    W2 = np.ascontiguousarray(np.asarray(W2, dtype=np.float32))
    b2 = np.ascontiguousarray(np.asarray(b2, dtype=np.float32))

    T = x.shape[0]
    TB = _tb()

    # Host-side routing + top-2 softmax gate weights (float64).
    logits = x.astype(np.float64) @ Wg.astype(np.float64) + bg.astype(np.float64)
    top2 = np.argpartition(logits, -2, axis=1)[:, -2:]
    l2 = np.take_along_axis(logits, top2, axis=1)          # [T, 2]
    m = l2.max(axis=1, keepdims=True)
    e2 = np.exp(l2 - m)
    sm = e2 / e2.sum(axis=1, keepdims=True)                # [T, 2]
    gatew = np.zeros((T, E), dtype=np.float64)
    gatew[np.arange(T)[:, None], top2] = sm                # dense [T, E]
    sel_mask = gatew > 0.0

    idx_e = [np.nonzero(sel_mask[:, e])[0] for e in range(E)]
    counts = [len(i) for i in idx_e]
    cap = max(P, int(math.ceil(max(counts) / P)) * P)
    nfull, rem = divmod(cap, TB)
    blocks = [TB] * nfull + ([rem] if rem else [])

    reps = int(os.environ.get("MOE_REPS", "1"))
    nc = _get_compiled(blocks, reps)

    import ml_dtypes
    npdt = {"f32r": np.float32, "f32": np.float32,
            "f16": np.float16, "bf16": ml_dtypes.bfloat16}[
        os.environ.get("MOE_DTYPE", "f16")]

    in_maps = []
    for e in range(E):
        n = counts[e]
        xe = np.zeros((cap, DIN), dtype=np.float32)
        xe[:n] = x[idx_e[e]]
        wv = np.zeros(cap, dtype=np.float32)
        wv[:n] = gatew[idx_e[e], e].astype(np.float32)
        in_maps.append({
            "xT": np.ascontiguousarray(
                xe.T.reshape(KC, P, cap).transpose(1, 0, 2).astype(npdt)),
            "W1m": np.ascontiguousarray(
                W1[e].reshape(KC, P, DH).transpose(1, 0, 2).astype(npdt)),
            "W2m": np.ascontiguousarray(
                W2[e].reshape(HC, P, DO).transpose(1, 0, 2).astype(npdt)),
            "b1c": np.ascontiguousarray(b1[e].reshape(HC, P).T),
            "b2t": np.ascontiguousarray(b2[e].reshape(DOC, P).T),
            "wvm": np.ascontiguousarray(np.tile(wv, (P, 1))),
        })

    _t0 = _time.time()
    res = run_bass_kernel_spmd(nc, in_maps, list(range(NCORES)))
    global LAST_DISPATCH_S
    LAST_DISPATCH_S = _time.time() - _t0

    outf = np.zeros((T, DO), dtype=np.float32)
    for e in range(E):
        o2 = res.results[e]["out2"]                  # [P, DOC, cap]
        oe = (o2.transpose(2, 1, 0).astype(np.float32)
              .reshape(cap, DO))                     # [cap, DO]
        outf[idx_e[e]] += oe[:counts[e]]
    return outf
